# revision 13
# baseline (speedup 1.0000x reference)
"""Trainium2 Bass kernel for nn_MultiHeadAttention_37039797961289 — v4.

MHA: B=1, S=4096, D=768, H=12, HD=64, fp32 in/out, bf16 internal.

Tensor-parallel over heads: 24 half-heads = (head, query-half). Core c<4
owns heads {3c,3c+1,3c+2} x queries 0:2048; core c>=4 the same heads x
queries 2048:4096. Each core projects Q/K/V for its 3 heads (K/V over
all 4096 keys), runs softmax(QK^T/8)V for its 2048 queries in 4 rounds
of 512 queries, then re-shards head->sequence with one AllToAll per
round (round r's block belongs to core (cc+1+r)%4 of the half; own
block last, no exchange). Out-proj contracts the gathered per-sender
feature blocks (all-reduce-free).

Per-core engine budget: ACT exp of 25.2M scores = 53us/round (the
steady-state wall; rounds 1-3 are ACT-bound). Round 0 is PE-bound
(~100us: sweep + K/V projection feeders). Collectives, normalize, and
gathers are fully decoupled from the PE/ACT critical path:
- accumulators copied PSUM->SBUF right at the AV stop (frees banks)
- denominator on partition 0 (ones at V col 0), fast approx reciprocal
- AllToAll with 8x-replicated input (dynamic-offset writes fault on
  this runtime; replication is the proven workaround), full-size dummy
  warmup so the first real A2A runs warm (~19us)
- gather bounces emitted inside the round loop so they land mid-round
- xt loaded chunk-contiguously across 3 engine DMA queues
"""

import os
import sys

sys.path.insert(0, "/opt/trn_rl_repo")

import numpy as np

import concourse.bass as bass
import concourse.mybir as mybir
import concourse.tile as tile
from concourse import bacc
from concourse.bass_utils import run_bass_kernel_spmd

FP32 = mybir.dt.float32
BF16 = mybir.dt.bfloat16
I32 = mybir.dt.int32
EXP = mybir.ActivationFunctionType.Exp

N_CORES = 8
D = 768
H = 12
HD = 64
S = 4096
KC = 6          # 128-row contraction chunks over D
QH = S // 2     # queries per core
NKB = S // 128  # 32 key blocks
NXC = 8         # xt column chunks (512 keys each)
RG = [list(range(N_CORES))]

NO_CC = bool(int(os.environ.get("K3_NO_CC", "0")))


def build_nc():
    nc = bacc.Bacc(None)

    xt = nc.dram_tensor("xt", [128, NXC, KC, 512], BF16, kind="ExternalInput")
    wq3 = nc.dram_tensor("wq3", [128, KC, 192], BF16, kind="ExternalInput")
    wk3 = nc.dram_tensor("wk3", [128, KC, 192], BF16, kind="ExternalInput")
    wv3 = nc.dram_tensor("wv3", [128, KC, 192], BF16, kind="ExternalInput")
    bq3 = nc.dram_tensor("bq3", [128, 2], FP32, kind="ExternalInput")
    bk3 = nc.dram_tensor("bk3", [128, 2], FP32, kind="ExternalInput")
    bv3 = nc.dram_tensor("bv3", [1, 192], FP32, kind="ExternalInput")
    wo8 = nc.dram_tensor("wo8", [128, 8, D], BF16, kind="ExternalInput")
    bo = nc.dram_tensor("bo", [1, D], FP32, kind="ExternalInput")
    rot = nc.dram_tensor("rot", [1, 8], I32, kind="ExternalInput")
    out = nc.dram_tensor("out", [512, D], FP32, kind="ExternalOutput")

    ab_in = [nc.dram_tensor(f"ab_in{k}", [8 * 128, 512], BF16) for k in range(3)]
    ab_out = [nc.dram_tensor(f"ab_out{k}", [8 * 128, 512], BF16) for k in range(3)]
    cc_in = [nc.dram_tensor(f"cc_in{k}", [8 * 64, 512], BF16) for k in range(3)]
    cc_out = [nc.dram_tensor(f"cc_out{k}", [8 * 64, 512], BF16) for k in range(3)]
    gab = [nc.dram_tensor(f"gab{k}", [8 * 128, 512], BF16) for k in range(3)]
    gcc = [nc.dram_tensor(f"gcc{k}", [8 * 64, 512], BF16) for k in range(3)]
    dum_in = nc.dram_tensor("dum_in", [8 * 128, 512], BF16)
    dum_out = nc.dram_tensor("dum_out", [8 * 128, 512], BF16)

    with tile.TileContext(nc) as tc:
        with tc.tile_pool(name="persist", bufs=1) as P:
            # ---- full-size dummy A2A: absorb collective setup AND warm
            # the 1.5MB size class (v2's first real A2A ran 2x slow) ----
            if not NO_CC:
                nc.gpsimd.collective_compute(
                    "AllToAll", mybir.AluOpType.bypass, replica_groups=RG,
                    ins=[dum_in[:]], outs=[dum_out[:]])

            # ---- rotation offsets -> SP registers ----
            rot_sb = P.tile([1, 8], I32, name="rot_sb")
            nc.sync.dma_start(rot_sb[:], rot[:])
            r_ab = []   # 128*(4*hf + (cc+1+j)%4)
            r_c = []    # 64*(4*hf + (cc+1+j)%4)
            for i in range(3):
                rg0 = nc.sync.alloc_register(f"rb{i}")
                nc.sync.reg_load(rg0, rot_sb[0:1, i : i + 1])
                r_ab.append(nc.sync.snap(rg0, donate=True, min_val=0,
                                         max_val=896))
                rg1 = nc.sync.alloc_register(f"rc{i}")
                nc.sync.reg_load(rg1, rot_sb[0:1, 4 + i : 5 + i])
                r_c.append(nc.sync.snap(rg1, donate=True, min_val=0,
                                        max_val=448))

            # ---- small resident tiles ----
            bq_sb = P.tile([128, 2], FP32, name="bq_sb")
            nc.sync.dma_start(bq_sb[:], bq3[:])
            bk_sb = P.tile([128, 2], FP32, name="bk_sb")
            nc.sync.dma_start(bk_sb[:], bk3[:])
            bv_sb = P.tile([1, 192], FP32, name="bv_sb")
            nc.sync.dma_start(bv_sb[:], bv3[:])
            bv_bc = P.tile([128, 192], FP32, name="bv_bc")
            nc.gpsimd.partition_broadcast(bv_bc[:], bv_sb[:])
            bo_sb = P.tile([1, D], FP32, name="bo_sb")
            nc.gpsimd.dma_start(bo_sb[:], bo[:])
            bo_bc = P.tile([128, D], FP32, name="bo_bc")
            nc.gpsimd.partition_broadcast(bo_bc[:], bo_sb[:])

            # ---- big resident tiles; xt chunks spread over 3 DMA queues
            # (sync/vector/scalar) so the load pipelines with compute ----
            wq_sb = P.tile([128, KC, 192], BF16, name="wq_sb")
            nc.sync.dma_start(wq_sb[:], wq3[:])
            wk_sb = P.tile([128, KC, 192], BF16, name="wk_sb")
            nc.sync.dma_start(wk_sb[:], wk3[:])
            wv_sb = P.tile([128, KC, 192], BF16, name="wv_sb")
            nc.scalar.dma_start(wv_sb[:], wv3[:])
            xt_sb = []
            xt_q = {0: nc.sync, 1: nc.scalar, 2: nc.sync, 3: nc.scalar,
                    4: nc.sync, 5: nc.gpsimd, 6: nc.gpsimd, 7: nc.gpsimd}
            for ci in range(NXC):
                t = P.tile([128, KC, 512], BF16, name=f"xt{ci}")
                xt_q[ci].dma_start(t[:], xt[:, ci])
                xt_sb.append(t)
            wo_sb = P.tile([128, 8, D], BF16, name="wo_sb")
            nc.gpsimd.dma_start(wo_sb[:], wo8[:])

            qt_ab = P.tile([128, QH], BF16, name="qt_ab")
            qt_c = P.tile([128, QH], BF16, name="qt_c")
            kt_ab = P.tile([128, S], BF16, name="kt_ab")
            kt_c = P.tile([128, S], BF16, name="kt_c")
            # v: col 0 = ones (softmax denominator lands on partition 0)
            v_sb = [P.tile([128, NKB, HD + 1], BF16, name=f"v{h}") for h in range(3)]
            for h in range(3):
                nc.vector.memset(v_sb[h][:, :, 0:1], 1.0)

            # out-proj lhsT: slots 0..3 = 128-row blocks of senders
            # (cc+1+j)%4 (slot 3 = own), slots 4..7 = their 64-row tails
            lhsT = P.tile([128, 8, 512], BF16, name="lhsT")

            # persistent normalize scratch (no pool rotation: avoids
            # alloc-wait cycles and keeps rounds decoupled)
            na_a = P.tile([HD + 1, 512], FP32, name="na_a")
            na_b = P.tile([HD + 1, 512], FP32, name="na_b")
            nco = P.tile([HD + 1, 512], FP32, name="nco")
            nsum = P.tile([HD + 1, 512], FP32, name="nsum")
            rc_a = P.tile([1, 512], FP32, name="rc_a")
            rc_b = P.tile([1, 512], FP32, name="rc_b")
            rc_c = P.tile([1, 512], FP32, name="rc_c")
            bc_a = P.tile([HD + 1, 512], FP32, name="bc_a")
            bc_b = P.tile([HD + 1, 512], FP32, name="bc_b")
            bc_c = P.tile([HD + 1, 512], FP32, name="bc_c")
            st_a = P.tile([HD + 1, 512], BF16, name="st_a")
            st_b = P.tile([HD + 1, 512], BF16, name="st_b")
            st_c = P.tile([HD + 1, 512], BF16, name="st_c")

            with tc.tile_pool(name="pA", bufs=2, space="PSUM") as pA, \
                 tc.tile_pool(name="scp", bufs=2, space="PSUM") as scp, \
                 tc.tile_pool(name="atp", bufs=2, space="PSUM") as atp, \
                 tc.tile_pool(name="ep", bufs=3) as ep, \
                 tc.tile_pool(name="smp", bufs=3) as smp:

                # ---------- phase A pieces ----------
                def proj_qk_half(w_sb, b_sb, dst_ab, dst_c, ci, dup):
                    """One 512-col chunk of Q^T/K^T (both feat groups)."""
                    proj_qk_pair(w_sb, b_sb, dst_ab, dst_c, (ci,), dup)

                def proj_qk_pair(w_sb, b_sb, dst_ab, dst_c, cis, dup):
                    """1-2 chunks of Q^T/K^T; chunks share each stationary
                    load (the PE reuses the loaded weights back-to-back)."""
                    ps1 = {}
                    for ci in cis:
                        ps1[ci] = pA.tile([128, 512], FP32, name="pa")
                    for k in range(KC):
                        for ci in cis:
                            nc.tensor.matmul(
                                ps1[ci][:], w_sb[:, k, 0:128],
                                xt_sb[ci][:, k, :],
                                start=(k == 0), stop=(k == KC - 1))
                    for ci in cis:
                        nc.vector.tensor_scalar_add(
                            out=dst_ab[:, 512 * ci : 512 * ci + 512],
                            in0=ps1[ci][:], scalar1=b_sb[:, 0:1])
                    ps2 = {}
                    for ci in cis:
                        ps2[ci] = pA.tile([128, 512], FP32, name="pa")
                    for k in range(KC):
                        for ci in cis:
                            nc.tensor.matmul(
                                ps2[ci][0:64, :], w_sb[:, k, 128:192],
                                xt_sb[ci][:, k, :],
                                start=(k == 0), stop=(k == KC - 1))
                    for ci in cis:
                        nc.vector.tensor_scalar_add(
                            out=dst_c[0:64, 512 * ci : 512 * ci + 512],
                            in0=ps2[ci][0:64, :], scalar1=b_sb[0:64, 1:2])
                        if dup:  # hC rows duplicated into partitions 64:128
                            nc.gpsimd.dma_start(
                                dst_c[64:128, 512 * ci : 512 * ci + 512],
                                dst_c[0:64, 512 * ci : 512 * ci + 512])

                def proj_v(st):
                    xtc = xt_sb[st // 4]
                    o = 128 * (st % 4)
                    ps = pA.tile([128, 512], FP32, name="pa")
                    for k in range(KC):
                        nc.tensor.matmul(
                            ps[:, 0:192], xtc[:, k, o : o + 128],
                            wv_sb[:, k, :],
                            start=(k == 0), stop=(k == KC - 1))
                    vsum = smp.tile([128, 192], FP32, name="vsum")
                    nc.vector.tensor_add(out=vsum[:], in0=ps[:, 0:192], in1=bv_bc[:])
                    for h in range(3):
                        nc.vector.tensor_copy(
                            out=v_sb[h][:, st, 1 : HD + 1],
                            in_=vsum[:, HD * h : HD * (h + 1)])

                # upfront (grouped by xt chunk): Q c0, K c0..1, V st0..13
                proj_qk_half(wq_sb, bq_sb, qt_ab, qt_c, 0, True)
                proj_qk_half(wk_sb, bk_sb, kt_ab, kt_c, 0, True)
                for st in range(0, 4):
                    proj_v(st)
                proj_qk_half(wk_sb, bk_sb, kt_ab, kt_c, 1, True)
                for st in range(4, 14):
                    proj_v(st)

                # feeders keyed by global sweep step (48 per round);
                # K chunks paired to share stationary loads, Q c2/c3
                # deferred into rounds 1-2 (deadline = round start)
                feeders = {}
                feeders[0] = lambda: proj_qk_pair(
                    wk_sb, bk_sb, kt_ab, kt_c, (2, 3), True)
                fv = {st: (lambda st=st: proj_v(st)) for st in range(14, NKB)}
                feeders[2] = fv[14]
                feeders[3] = fv[15]
                feeders[4] = lambda: proj_qk_pair(
                    wk_sb, bk_sb, kt_ab, kt_c, (4, 5), True)
                for i, st in enumerate(range(16, 22)):
                    feeders[6 + i] = fv[st]
                feeders[12] = lambda: proj_qk_pair(
                    wk_sb, bk_sb, kt_ab, kt_c, (6, 7), True)
                for i, st in enumerate(range(22, NKB)):
                    feeders[14 + i] = fv[st]
                feeders[26] = lambda: proj_qk_half(
                    wq_sb, bq_sb, qt_ab, qt_c, 1, True)
                feeders[50] = lambda: proj_qk_half(
                    wq_sb, bq_sb, qt_ab, qt_c, 2, True)
                feeders[98] = lambda: proj_qk_half(
                    wq_sb, bq_sb, qt_ab, qt_c, 3, True)

                def gather(j):
                    """lhsT slots j / 4+j <- exchanged blocks (round 2-j)."""
                    if NO_CC:
                        for ch in range(8):
                            nc.sync.dma_start(
                                gab[2 - j][128 * ch : 128 * (ch + 1), :],
                                ab_in[2 - j][0:128, :])
                            nc.sync.dma_start(
                                gcc[2 - j][64 * ch : 64 * (ch + 1), :],
                                cc_in[2 - j][0:64, :])
                    else:
                        nc.sync.dma_start(gab[2 - j][:], ab_out[2 - j][:])
                        nc.sync.dma_start(gcc[2 - j][:], cc_out[2 - j][:])
                    nc.sync.dma_start(
                        lhsT[:, j, :], gab[2 - j][bass.ds(r_ab[j], 128), :])
                    nc.sync.dma_start(
                        lhsT[0:64, 4 + j, :],
                        gcc[2 - j][bass.ds(r_c[j], 64), :])

                # ---------- phase B: 4 rounds ----------
                for r in range(4):
                    qsl = 512 * r  # host pre-rotates query columns
                    # gathers land mid-round, off the critical path
                    if r == 2:
                        gather(2)   # cc_out[0]: A2A 0 done mid round 1
                    elif r == 3:
                        gather(1)   # cc_out[1]: A2A 1 done mid round 2
                        gather(0)   # cc_out[2]: A2A 2 done mid round 3

                    # -- dual sweep: hA rows 0:64, hB rows 64:128 --
                    at_a = atp.tile([HD + 1, 512], FP32, name="at")
                    at_b = atp.tile([HD + 1, 512], FP32, name="at")
                    prev = None
                    for kb in range(NKB):
                        sc = scp.tile([128, 1024], FP32, name="sc")
                        for s_i, pr in ((0, slice(0, 64)), (1, slice(64, 128))):
                            nc.tensor.matmul(
                                sc[:, 512 * s_i : 512 * (s_i + 1)],
                                kt_ab[pr, 128 * kb : 128 * (kb + 1)],
                                qt_ab[pr, qsl : qsl + 512],
                                start=True, stop=True)
                        gstep = 48 * r + kb
                        if gstep in feeders:
                            feeders.pop(gstep)()
                        e = ep.tile([128, 1024], BF16, name="e")
                        nc.scalar.activation(e[:], sc[:], EXP)
                        if prev is not None:
                            pe, pkb = prev
                            nc.tensor.matmul(
                                at_a[:], v_sb[0][:, pkb, :], pe[:, 0:512],
                                start=(pkb == 0), stop=False)
                            nc.tensor.matmul(
                                at_b[:], v_sb[1][:, pkb, :], pe[:, 512:1024],
                                start=(pkb == 0), stop=False)
                        prev = (e, kb)
                    pe, pkb = prev
                    nc.tensor.matmul(at_a[:], v_sb[0][:, pkb, :], pe[:, 0:512],
                                     start=False, stop=True)
                    nc.tensor.matmul(at_b[:], v_sb[1][:, pkb, :], pe[:, 512:1024],
                                     start=False, stop=True)

                    # -- decouple: PSUM -> SBUF copies free the banks --
                    nc.vector.tensor_copy(out=na_a[:], in_=at_a[:])
                    nc.vector.tensor_copy(out=na_b[:], in_=at_b[:])

                    # -- normalize a/b from SBUF (den on partition 0) --
                    for na, rc, bc, stt in (
                        (na_a, rc_a, bc_a, st_a), (na_b, rc_b, bc_b, st_b)):
                        nc.vector.reciprocal_approx_fast(rc[:], na[0:1, :])
                        nc.gpsimd.partition_broadcast(bc[:], rc[:])
                        nc.vector.tensor_mul(
                            out=stt[:], in0=na[:], in1=bc[:])
                    if r < 3:
                        for ch in range(8):
                            nc.gpsimd.dma_start(
                                ab_in[r][128 * ch : 128 * ch + 64, :],
                                st_a[1 : HD + 1, :])
                            nc.gpsimd.dma_start(
                                ab_in[r][128 * ch + 64 : 128 * (ch + 1), :],
                                st_b[1 : HD + 1, :])
                        if not NO_CC:
                            nc.gpsimd.collective_compute(
                                "AllToAll", mybir.AluOpType.bypass,
                                replica_groups=RG,
                                ins=[ab_in[r][:]], outs=[ab_out[r][:]])
                    else:
                        nc.gpsimd.dma_start(lhsT[0:64, 3, :], st_a[1 : HD + 1, :])
                        nc.gpsimd.dma_start(lhsT[64:128, 3, :], st_b[1 : HD + 1, :])

                    # -- self sweep: hC, even kbs rows 0:64, odd rows 64:128 --
                    at_ce = atp.tile([HD + 1, 512], FP32, name="at")
                    at_co = atp.tile([HD + 1, 512], FP32, name="at")
                    prev2 = None
                    for kbp in range(NKB // 2):
                        kbe, kbo = 2 * kbp, 2 * kbp + 1
                        sc2 = scp.tile([128, 1024], FP32, name="sc")
                        nc.tensor.matmul(
                            sc2[:, 0:512],
                            kt_c[0:64, 128 * kbe : 128 * (kbe + 1)],
                            qt_c[0:64, qsl : qsl + 512],
                            start=True, stop=True)
                        nc.tensor.matmul(
                            sc2[:, 512:1024],
                            kt_c[64:128, 128 * kbo : 128 * (kbo + 1)],
                            qt_c[64:128, qsl : qsl + 512],
                            start=True, stop=True)
                        gstep = 48 * r + 32 + kbp
                        if gstep in feeders:
                            feeders.pop(gstep)()
                        e2 = ep.tile([128, 1024], BF16, name="e2")
                        nc.scalar.activation(e2[:], sc2[:], EXP)
                        if prev2 is not None:
                            p2, pk = prev2
                            nc.tensor.matmul(
                                at_ce[:], v_sb[2][:, 2 * pk, :],
                                p2[:, 0:512], start=(pk == 0), stop=False)
                            nc.tensor.matmul(
                                at_co[:], v_sb[2][:, 2 * pk + 1, :],
                                p2[:, 512:1024], start=(pk == 0), stop=False)
                        prev2 = (e2, kbp)
                    p2, pk = prev2
                    nc.tensor.matmul(at_ce[:], v_sb[2][:, 2 * pk, :],
                                     p2[:, 0:512], start=False, stop=True)
                    nc.tensor.matmul(at_co[:], v_sb[2][:, 2 * pk + 1, :],
                                     p2[:, 512:1024], start=False, stop=True)

                    # merge even/odd (row 0 = denominators merge for free)
                    nc.vector.tensor_copy(out=nco[:], in_=at_co[:])
                    nc.vector.tensor_add(out=nsum[:], in0=at_ce[:], in1=nco[:])
                    nc.vector.reciprocal_approx_fast(rc_c[:], nsum[0:1, :])
                    nc.gpsimd.partition_broadcast(bc_c[:], rc_c[:])
                    nc.vector.tensor_mul(
                        out=st_c[:], in0=nsum[:], in1=bc_c[:])
                    if r < 3:
                        for ch in range(8):
                            nc.gpsimd.dma_start(
                                cc_in[r][64 * ch : 64 * (ch + 1), :],
                                st_c[1 : HD + 1, :])
                        if not NO_CC:
                            nc.gpsimd.collective_compute(
                                "AllToAll", mybir.AluOpType.bypass,
                                replica_groups=RG,
                                ins=[cc_in[r][:]], outs=[cc_out[r][:]])
                    else:
                        nc.gpsimd.dma_start(lhsT[0:64, 7, :], st_c[1 : HD + 1, :])

                for k in sorted(feeders):
                    feeders.pop(k)()

            # ---- output projection, outside the sweep PSUM pools:
            # 4 t-groups x N=768 (1.5 banks each); the 24 peer-block
            # matmuls (slots 0-2/4-6, ready mid-round-3) run first and
            # keep the PE warm while the own normalize lands slots 3/7 ----
            with tc.tile_pool(name="opj", bufs=8, space="PSUM") as opj, \
                 tc.tile_pool(name="oev", bufs=4) as oev:
                ps_g = {}
                for t in range(4):
                    for n in range(2):
                        nsl = slice(384 * n, 384 * (n + 1))
                        ps_full = opj.tile([128, 512], FP32, name="op")
                        ps_t = ps_full[:, 0:384]
                        ps_g[(t, n)] = ps_t
                        for gi, g in enumerate((0, 1, 2, 4, 5, 6)):
                            rows = slice(0, 128 if g < 4 else 64)
                            nc.tensor.matmul(
                                ps_t, lhsT[rows, g, 128 * t : 128 * (t + 1)],
                                wo_sb[rows, g, nsl],
                                start=(gi == 0), stop=False)
                for t in range(4):
                    for n in range(2):
                        nsl = slice(384 * n, 384 * (n + 1))
                        ps_t = ps_g[(t, n)]
                        nc.tensor.matmul(
                            ps_t, lhsT[:, 3, 128 * t : 128 * (t + 1)],
                            wo_sb[:, 3, nsl], start=False, stop=False)
                        nc.tensor.matmul(
                            ps_t, lhsT[0:64, 7, 128 * t : 128 * (t + 1)],
                            wo_sb[0:64, 7, nsl], start=False, stop=True)
                        o_ev = oev.tile([128, 384], FP32, name="o_ev")
                        nc.vector.tensor_add(
                            out=o_ev[:], in0=ps_t, in1=bo_bc[:, nsl])
                        oq = nc.sync if n == 0 else nc.gpsimd
                        oq.dma_start(
                            out[128 * t : 128 * (t + 1), nsl], o_ev[:])

    nc.finalize()
    return nc


_NC_CACHE = None


def _get_nc():
    global _NC_CACHE
    if _NC_CACHE is None:
        _NC_CACHE = build_nc()
    return _NC_CACHE


def make_in_maps(hidden_states, Wq, Wk, Wv, bq, bk, bv, Wo, bo):
    import ml_dtypes

    bf16 = ml_dtypes.bfloat16
    x = np.asarray(hidden_states, dtype=np.float32)[0]  # [S, D]
    scale = np.float32(1.0 / np.sqrt(np.float32(HD)))

    Wq = np.asarray(Wq, np.float32) * scale  # [H, D, HD]
    Wk = np.asarray(Wk, np.float32)
    Wv = np.asarray(Wv, np.float32)
    bq_s = (np.asarray(bq, np.float32) * scale).reshape(H, HD)
    bk_r = np.asarray(bk, np.float32).reshape(H, HD)
    bv_r = np.asarray(bv, np.float32).reshape(H, HD)
    Wo_r = np.asarray(Wo, np.float32)  # [D, D]
    bo_r = np.asarray(bo, np.float32).reshape(1, D)

    xT = np.ascontiguousarray(x.T)  # [D, S]
    xt_full = xT.reshape(KC, 128, S).transpose(1, 0, 2)  # [128, KC, S]

    in_maps = []
    for c in range(N_CORES):
        hf, cc = c // 4, c % 4
        heads = [3 * cc, 3 * cc + 1, 3 * cc + 2]
        qown = slice(QH * hf, QH * hf + QH)
        qother = slice(QH, S) if hf == 0 else slice(0, QH)

        # own-half query columns rotated so round r computes the block
        # destined for core (cc+1+r)%4; keys use the same (rotated)
        # order for K and V, which is softmax-invariant.
        own = xt_full[:, :, qown]
        perm = [(cc + 1 + r) % 4 for r in range(4)]
        own_rot = np.concatenate(
            [own[:, :, 512 * p : 512 * (p + 1)] for p in perm], axis=2)
        xt_c = np.concatenate([own_rot, xt_full[:, :, qother]], axis=2)
        # chunk-contiguous layout: [128, NXC, KC, 512]
        xt_4d = np.ascontiguousarray(
            xt_c.reshape(128, KC, NXC, 512).transpose(0, 2, 1, 3))

        def wstack(W):  # [H, D, HD] -> [128, KC, 192]
            w = np.concatenate([W[h] for h in heads], axis=1)  # [D, 192]
            return np.ascontiguousarray(
                w.reshape(KC, 128, 192).transpose(1, 0, 2)).astype(bf16)

        def bstack(b):
            flat = np.concatenate([b[h] for h in heads])
            col = np.zeros((128, 2), np.float32)
            col[:, 0] = flat[0:128]
            col[0:64, 1] = flat[128:192]
            return col

        # wo8: slot j (0..3) = sender sigma_j = (cc+1+j)%4 of my half
        # (slot 3 = own cc); Wo rows 192*sigma_j .. +128.
        # slot 4+j = rows +128 .. +192 (only partitions 0:64 used).
        wo_rows = np.zeros((8, 128, D), np.float32)
        for j in range(4):
            sg = (cc + 1 + j) % 4
            wo_rows[j] = Wo_r[192 * sg : 192 * sg + 128]
            wo_rows[4 + j][0:64] = Wo_r[192 * sg + 128 : 192 * (sg + 1)]
        wo8 = np.ascontiguousarray(
            wo_rows.transpose(1, 0, 2)).astype(bf16)

        rot_off = np.array(
            [[128 * (4 * hf + (cc + 1 + i) % 4) for i in range(3)] + [0]
             + [64 * (4 * hf + (cc + 1 + i) % 4) for i in range(3)] + [0]],
            dtype=np.int32)

        in_maps.append({
            "xt": xt_4d.astype(bf16),
            "wq3": wstack(Wq), "wk3": wstack(Wk), "wv3": wstack(Wv),
            "bq3": bstack(bq_s), "bk3": bstack(bk_r),
            "bv3": np.concatenate(
                [bv_r[h] for h in heads]).reshape(1, 192).astype(np.float32),
            "wo8": wo8, "bo": bo_r, "rot": rot_off,
        })
    return in_maps


def kernel(hidden_states, Wq, Wk, Wv, bq, bk, bv, Wo, bo):
    in_maps = make_in_maps(hidden_states, Wq, Wk, Wv, bq, bk, bv, Wo, bo)
    nc = _get_nc()
    last_err = None
    for _attempt in range(3):
        try:
            res = run_bass_kernel_spmd(nc, in_maps, list(range(N_CORES)))
            break
        except Exception as e:
            last_err = e
            import time

            time.sleep(2.0)
    else:
        raise last_err
    outs = [res.results[c]["out"] for c in range(N_CORES)]
    return np.concatenate(outs, axis=0)[None, :, :].astype(np.float32)
